# revision 25
# baseline (speedup 1.0000x reference)
"""Trainium2 Bass kernel for AttentionForONNX decode-path self-attention.

Problem shapes (hardcoded): T=4, B=32, E=1024, H=16, HD=64, CACHE=4096, S=4100.
Sharding: batch B=32 split across 8 cores (BL=4 batches/core). Each core runs
the full attention for its 4 batches x 16 heads independently (no collectives);
host concatenates outputs on B.

Host-side prep (part of the sharding step): K cache is uploaded pre-transposed
and pre-tiled in bf16 (head-pairs interleaved on partitions), V cache bf16
pre-tiled to match the score chunk layout, weights uploaded as W^T bf16 tiles,
x as x^T bf16 tiles, and the key-padding mask as a pre-broadcast multiplicative
bf16 mask. This halves HBM traffic vs fp32 and removes all on-chip transposes
of the large operands.

Per-core kernel (memory-bound; K+V caches = 67MB/core dominate):
  - Q/K projections computed transposed (lhsT = W^T chunk, rhs = x^T chunk) so
    q^T/k_new^T land hd-major with no extra transposes; bias added via a fused
    DVE tensor_scalar_add on the PSUM->SBUF copy. V/out projections computed
    natural with a ones-row bias matmul.
  - Main loop over 32 (batch, head-pair) groups: one 1MB DMA for K^T of two
    heads [128=2*hd, 4096=s], one for V of two heads; 16 score matmuls
    (lhsT=K^T chunk [128,128], rhs=stacked q [128,8]) + 2 tail matmuls; one
    Exp ACT over [128, 264] (scale=0.125 folds the 1/sqrt(HD)); one DVE
    multiply applies the key-padding mask multiplicatively (masked prob = 0);
    64+2 PV matmuls accumulate O natural [4, 64] per head; a ones-column
    matmul gives Z; DVE reduce/reciprocal/transpose produce 1/Z per-partition
    and a tensor_scalar_mul writes normalized O.
"""

import numpy as np

T, B, E = 4, 32, 1024
H, HD = 16, 64
CACHE = 4096
S = CACHE + T
NCORES = 8
BL = B // NCORES  # batches per core = 4
ROWS = T * BL  # 16 projection rows per core, (b, t) order
NHP = H // 2  # 8 head-pairs
NGRP = BL * NHP  # 32 (b, head-pair) groups per core
NCH = CACHE // 128  # 32 s-chunks of 128
SCW = 8 * NCH + 8  # 264 score cols: (c, g, t) main + tail block


def build_bass():
    import concourse.bass as bass
    import concourse.bacc as bacc
    import concourse.mybir as mybir
    from concourse.masks import make_identity
    from concourse.tile import TileContext

    f32 = mybir.dt.float32
    bf16 = mybir.dt.bfloat16
    AF = mybir.ActivationFunctionType

    nc = bacc.Bacc(None)

    # ---- DRAM inputs (host pre-tiled, bf16 unless noted) ----
    xt_t = nc.dram_tensor("xt_t", [128, 8 * ROWS], bf16, kind="ExternalInput")
    kvt = nc.dram_tensor("kvt", [BL, NHP, 128, 8192], bf16, kind="ExternalInput")
    maskt = nc.dram_tensor("maskt", [BL, 128, SCW], bf16, kind="ExternalInput")
    wqt = nc.dram_tensor("wqt", [128, 8192], bf16, kind="ExternalInput")
    wkt = nc.dram_tensor("wkt", [128, 8192], bf16, kind="ExternalInput")
    wvt = nc.dram_tensor("wvt", [128, 8192], bf16, kind="ExternalInput")
    wot = nc.dram_tensor("wot", [128, 8192], bf16, kind="ExternalInput")
    bqt = nc.dram_tensor("bqt", [64, H], f32, kind="ExternalInput")
    bkt = nc.dram_tensor("bkt", [64, H], f32, kind="ExternalInput")
    bv_b = nc.dram_tensor("bv_b", [1, E], bf16, kind="ExternalInput")
    bo_b = nc.dram_tensor("bo_b", [1, E], bf16, kind="ExternalInput")
    out = nc.dram_tensor("out", [ROWS, E], f32, kind="ExternalOutput")

    with TileContext(nc) as tc:
        with (
            tc.tile_pool(name="const", bufs=1) as constp,
            tc.tile_pool(name="wts", bufs=1) as wtsp,
            tc.tile_pool(name="kv", bufs=3) as kvp,
            tc.tile_pool(name="ptp", bufs=2) as ptp,
            tc.tile_pool(name="ztp", bufs=2) as ztp,
            tc.tile_pool(name="ps_sc", bufs=2, space="PSUM") as ps_sc,
            tc.tile_pool(name="ps_pv", bufs=3, space="PSUM") as ps_pv,
            tc.tile_pool(name="ps_pj1", bufs=1, space="PSUM") as ps_pj1,
            tc.tile_pool(name="ps_pj2", bufs=1, space="PSUM") as ps_pj2,
        ):
            # ---- startup loads, ordered so q-projection can start ASAP ----
            wq_sb = wtsp.tile([128, 8192], bf16, tag="wq")
            nc.sync.dma_start(out=wq_sb[:, :], in_=wqt[:, :])
            xt = constp.tile([128, 8 * ROWS], bf16, tag="xt")
            nc.sync.dma_start(out=xt[:, :], in_=xt_t[:, :])
            bq_sb = constp.tile([64, H], f32, tag="bq")
            nc.sync.dma_start(out=bq_sb[:, :], in_=bqt[:, :])
            wk_sb = wtsp.tile([128, 8192], bf16, tag="wk")
            nc.sync.dma_start(out=wk_sb[:, :], in_=wkt[:, :])
            bk_sb = constp.tile([64, H], f32, tag="bk")
            nc.sync.dma_start(out=bk_sb[:, :], in_=bkt[:, :])
            wv_sb = wtsp.tile([128, 8192], bf16, tag="wv")
            nc.sync.dma_start(out=wv_sb[:, :], in_=wvt[:, :])
            bv_sb = constp.tile([1, E], bf16, tag="bv")
            nc.sync.dma_start(out=bv_sb[:, :], in_=bv_b[:, :])
            mask_sb = constp.tile([128, BL * SCW], bf16, tag="mask")
            for b in range(BL):
                nc.sync.dma_start(
                    out=mask_sb[:, SCW * b : SCW * (b + 1)], in_=maskt[b]
                )
            bo_sb = constp.tile([1, E], bf16, tag="bo")
            nc.sync.dma_start(out=bo_sb[:, :], in_=bo_b[:, :])
            wo_sb = wtsp.tile([128, 8192], bf16, tag="wo")
            nc.sync.dma_start(out=wo_sb[:, :], in_=wot[:, :])

            # ---- constants ----
            ones_col = constp.tile([128, 1], bf16, tag="ones_col")
            nc.vector.memset(ones_col[:, :], 1.0)
            ones_row = constp.tile([1, ROWS], bf16, tag="ones_row")
            nc.vector.memset(ones_row[:, :], 1.0)
            ones_r64 = constp.tile([1, 64], f32, tag="ones_r64")
            nc.vector.memset(ones_r64[:, :], 1.0)

            # ---- transposed q/k projections: pT[64, 16(h)*16(b,t)] ----
            # wq_sb layout: [:, (c*16 + h)*64 : +64] = W^T rows e-chunk c, cols
            # j in [64h, 64h+64).  psum [64, 16] per h accumulated over c.
            def projT(w_sb, bias_sb, dest):
                pj = ps_pj1.tile([128, 16 * H], f32, tag="pj1")
                for h in range(H):
                    for c in range(8):
                        nc.tensor.matmul(
                            pj[0:64, 16 * h : 16 * (h + 1)],
                            w_sb[:, (c * 16 + h) * 64 : (c * 16 + h) * 64 + 64],
                            xt[:, ROWS * c : ROWS * (c + 1)],
                            start=(c == 0),
                            stop=(c == 7),
                        )
                for h in range(H):
                    nc.vector.tensor_scalar_add(
                        dest[0:64, 16 * h : 16 * (h + 1)],
                        pj[0:64, 16 * h : 16 * (h + 1)],
                        bias_sb[0:64, h : h + 1],
                    )

            qT = constp.tile([64, 16 * H], bf16, tag="qT")
            projT(wq_sb, bq_sb, qT)
            kT = constp.tile([64, 16 * H], bf16, tag="kT")
            projT(wk_sb, bk_sb, kT)

            # q duplicated on partitions 64:128 (SBUF->SBUF DMA partition move)
            qdup = constp.tile([128, 16 * H], bf16, tag="qdup")
            nc.sync.dma_start(out=qdup[64:128, :], in_=qT[0:64, :])

            # q2_stack [128, 8*NGRP]: group g=(b*NHP+hp): rows 0:64 cols 8g+0:4
            # = q^T(b, 2hp); rows 64:128 cols 8g+4:8 = q^T(b, 2hp+1)
            q2s = constp.tile([128, 8 * NGRP], bf16, tag="q2s")
            nc.vector.memset(q2s[:, :], 0.0)
            q2s_top = q2s[0:64, :].rearrange("p (b r) -> p b r", r=8 * NHP)
            q2s_bot = q2s[64:128, :].rearrange("p (b r) -> p b r", r=8 * NHP)
            for hp in range(NHP):
                # src cols for head h: 16h + 4b + t ; dst cols 8*(b*8+hp)+...
                nc.vector.tensor_copy(
                    q2s_top[:, :, 8 * hp : 8 * hp + 4],
                    qT[0:64, 16 * (2 * hp) : 16 * (2 * hp) + 16]
                    .rearrange("p (b t) -> p b t", t=T),
                )
                nc.vector.tensor_copy(
                    q2s_bot[:, :, 8 * hp + 4 : 8 * hp + 8],
                    qdup[64:128, 16 * (2 * hp + 1) : 16 * (2 * hp + 1) + 16]
                    .rearrange("p (b t) -> p b t", t=T),
                )

            # knt2p [64, H*128]: head h block cols 128h:128h+128, cols 0:16 =
            # k_new^T (b', t'), rest zero (pads tail-score out to 128 rows)
            knt2p = constp.tile([64, H * 128], bf16, tag="knt2p")
            nc.vector.memset(knt2p[:, :], 0.0)
            for h in range(H):
                nc.vector.tensor_copy(
                    knt2p[0:64, 128 * h : 128 * h + 16],
                    kT[0:64, 16 * h : 16 * (h + 1)],
                )

            # ---- natural v projection: vn [16, 1024] bf16 ----
            pj2 = ps_pj2.tile([ROWS, E], f32, tag="pj2")
            for half in range(2):
                sl = slice(512 * half, 512 * (half + 1))
                for c in range(8):
                    nc.tensor.matmul(
                        pj2[:, sl],
                        xt[:, ROWS * c : ROWS * (c + 1)],
                        wv_sb[:, 1024 * c + 512 * half : 1024 * c + 512 * (half + 1)],
                        start=(c == 0),
                        stop=False,
                    )
                nc.tensor.matmul(
                    pj2[:, sl], ones_row[:, :], bv_sb[:, sl], start=False, stop=True
                )
            vn = constp.tile([ROWS, E], bf16, tag="vn")
            nc.vector.tensor_copy(vn[:, :], pj2[:, :])

            # ---- O^T accumulator: rows (h%2)*64+hd, cols ROWS*hp + 4b + t ----
            ot = constp.tile([128, 8 * ROWS], bf16, tag="ot")

            def stage_a(b, hp, pt, v2, pv):
                # PV (transposed output: O^T via lhsT=v chunk) + Z matmuls +
                # Z reduce/reciprocal; emitted one iteration late so these PE
                # matmuls fill the exp/mask bubble after the next group's
                # score matmuls.
                for gg in range(2):
                    for c in range(NCH):
                        nc.tensor.matmul(
                            pv[64 * gg : 64 * (gg + 1), 0:T],
                            v2[:, 2048 * gg + 64 * c : 2048 * gg + 64 * (c + 1)],
                            pt[:, 8 * c + 4 * gg : 8 * c + 4 * (gg + 1)],
                            start=(c == 0),
                            stop=False,
                        )
                    h = 2 * hp + gg
                    nc.tensor.matmul(
                        pv[64 * gg : 64 * (gg + 1), 0:T],
                        vn[:, 64 * h : 64 * (h + 1)],
                        pt[0:ROWS, 8 * NCH + 4 * gg : 8 * NCH + 4 * (gg + 1)],
                        start=False,
                        stop=True,
                    )
                # Z row-sums via ones-column matmul -> [1, 264]
                nc.tensor.matmul(
                    pv[0:1, 128 : 128 + SCW],
                    ones_col[:, :],
                    pt[:, :],
                    start=True,
                    stop=True,
                )
                zt = ztp.tile([1, 16], f32, tag="zt")
                nc.vector.reduce_sum(
                    zt[0:1, 8:16],
                    pv[0:1, 128 : 128 + SCW].rearrange("p (c x) -> p x c", x=8),
                    axis=mybir.AxisListType.X,
                )
                nc.vector.reciprocal(zt[0:1, 0:8], zt[0:1, 8:16])
                return zt

            def stage_b(b, hp, pv, zt):
                # broadcast 1/Z over the hd partitions (fp32 PE matmul) and
                # write the normalized O^T block; two stages behind the
                # scores so the PE never waits on the Z chain.
                for gg in range(2):
                    nc.tensor.matmul(
                        pv[64 * gg : 64 * (gg + 1), 256:260],
                        ones_r64[:, :],
                        zt[0:1, 4 * gg : 4 * (gg + 1)],
                        start=True,
                        stop=True,
                    )
                zbs = ztp.tile([128, T], bf16, tag="zbs")
                nc.scalar.activation(zbs[:, :], pv[:, 256:260], AF.Copy)
                nc.vector.tensor_mul(
                    ot[:, ROWS * hp + T * b : ROWS * hp + T * (b + 1)],
                    pv[:, 0:T],
                    zbs[:, :],
                )

            # ---- main attention loop over 32 groups (software-pipelined) ----
            pend_a = None
            pend_b = None
            for b in range(BL):
                for hp in range(NHP):
                    g = b * NHP + hp
                    kv2 = kvp.tile([128, 8192], bf16, tag="kv2")
                    nc.sync.dma_start(out=kv2[:, :], in_=kvt[b, hp])
                    kt2 = kv2[:, 0:CACHE]
                    v2 = kv2[:, CACHE:8192]

                    sc = ps_sc.tile([128, SCW], f32, tag="sc")
                    # main scores: S^T[s=128c+p, (g,t)] for both heads
                    for c in range(NCH):
                        nc.tensor.matmul(
                            sc[:, 8 * c : 8 * (c + 1)],
                            kt2[:, 128 * c : 128 * (c + 1)],
                            q2s[:, 8 * g : 8 * (g + 1)],
                            start=True,
                            stop=True,
                        )
                    # tail scores: rows (b', t'), own-b rows kept by the mask
                    for gg in range(2):
                        h = 2 * hp + gg
                        nc.tensor.matmul(
                            sc[:, 8 * NCH + 4 * gg : 8 * NCH + 4 * (gg + 1)],
                            knt2p[:, 128 * h : 128 * (h + 1)],
                            qT[0:64, 16 * h + 4 * b : 16 * h + 4 * b + 4],
                            start=True,
                            stop=True,
                        )

                    # P = exp(S/8) * mask, emitted BEFORE the delayed stages
                    # so the mask multiply is never queued behind the (PE-
                    # blocked) Z-chain ops in the strict-FIFO DVE queue.
                    pt_raw = ptp.tile([128, SCW], bf16, tag="pt_raw")
                    nc.scalar.activation(pt_raw[:, :], sc[:, :], AF.Exp, scale=0.125)
                    pt = ptp.tile([128, SCW], bf16, tag="pt")
                    nc.vector.tensor_mul(
                        pt[:, :], pt_raw[:, :], mask_sb[:, SCW * b : SCW * (b + 1)]
                    )
                    pv = ps_pv.tile([128, 512], f32, tag="pv")

                    next_b = None
                    if pend_a is not None:
                        zt = stage_a(*pend_a)
                        next_b = (pend_a[0], pend_a[1], pend_a[4], zt)
                    if pend_b is not None:
                        stage_b(*pend_b)
                    pend_b = next_b
                    pend_a = (b, hp, pt, v2, pv)
            zt = stage_a(*pend_a)
            if pend_b is not None:
                stage_b(*pend_b)
            stage_b(pend_a[0], pend_a[1], pend_a[4], zt)

            # ---- out projection ----
            out_ps = ps_pj2.tile([ROWS, E], f32, tag="pj2")
            for half in range(2):
                sl = slice(512 * half, 512 * (half + 1))
                for c in range(8):
                    nc.tensor.matmul(
                        out_ps[:, sl],
                        ot[:, ROWS * c : ROWS * (c + 1)],
                        wo_sb[:, 1024 * c + 512 * half : 1024 * c + 512 * (half + 1)],
                        start=(c == 0),
                        stop=False,
                    )
                nc.tensor.matmul(
                    out_ps[:, sl], ones_row[:, :], bo_sb[:, sl], start=False, stop=True
                )
            out_sb = constp.tile([ROWS, E], f32, tag="outsb")
            nc.vector.tensor_copy(out_sb[:, :], out_ps[:, :])
            nc.sync.dma_start(out=out[:, :], in_=out_sb[:, :])

    nc.finalize()
    return nc


_nc_cache = None
TRACE = False
LAST_RESULTS = None


def kernel(**inputs):
    global _nc_cache, LAST_RESULTS
    from concourse.bass_utils import run_bass_kernel_spmd
    import ml_dtypes

    bft = ml_dtypes.bfloat16

    query = np.asarray(inputs["query"], dtype=np.float32)
    mask = np.asarray(inputs["key_padding_mask"])
    kc = np.asarray(inputs["self_p_k"], dtype=np.float32)
    vc = np.asarray(inputs["self_p_v"], dtype=np.float32)

    # one packed [128, 8192] tile per (b, head-pair): cols 0:4096 = K^T with
    # the two heads stacked on partitions, cols 4096:8192 = V pre-tiled so
    # head g chunk c (cols 64c:64c+64) holds v rows s=128c+p
    kvt_all = np.empty((B, NHP, 128, 8192), dtype=bft)
    kvt_all[:, :, :, :CACHE] = (
        kc.astype(bft).reshape(B, NHP, 2, CACHE, HD).transpose(0, 1, 2, 4, 3)
    ).reshape(B, NHP, 128, CACHE)
    kvt_all[:, :, :, CACHE:] = (
        vc.astype(bft).reshape(B, NHP, 2, NCH, 128, HD).transpose(0, 1, 4, 2, 3, 5)
    ).reshape(B, NHP, 128, 2 * 2048)

    # multiplicative mask, pre-broadcast to the score layout [B, 128, SCW]
    minv = (~mask).astype(np.float32)  # [B, S]: 1 keep, 0 drop
    mm = np.zeros((B, 128, SCW), dtype=np.float32)
    main = minv[:, :CACHE].reshape(B, NCH, 128).transpose(0, 2, 1)  # [B, 128, c]
    mm[:, :, : 8 * NCH] = np.repeat(main, 8, axis=2)
    tail = minv[:, CACHE:]  # [B, T]
    for b in range(B):
        bl = b % BL  # local batch index on its core
        for j in range(T):
            for gg in range(2):
                for t in range(T):
                    mm[b, 4 * bl + j, 8 * NCH + 4 * gg + t] = tail[b, j]
    mm = mm.astype(bft)

    def wT_tiles_T(w):  # for transposed projections (lhsT layout)
        wt = w.astype(bft).T  # [e, j]
        return np.ascontiguousarray(
            wt.reshape(8, 128, H, 64).transpose(1, 0, 2, 3).reshape(128, 8192)
        )

    def wT_tiles_N(w):  # for natural projections (rhs layout)
        wt = w.astype(bft).T  # [e, j]
        return np.ascontiguousarray(
            wt.reshape(8, 128, E).transpose(1, 0, 2).reshape(128, 8192)
        )

    # note: the 1/sqrt(HD) q-scaling is folded into the on-chip exp scale
    wt_tiles = {
        "wqt": wT_tiles_T(np.asarray(inputs["Wq"], np.float32)),
        "wkt": wT_tiles_T(np.asarray(inputs["Wk"], np.float32)),
        "wvt": wT_tiles_N(np.asarray(inputs["Wv"], np.float32)),
        "wot": wT_tiles_N(np.asarray(inputs["Wo"], np.float32)),
    }
    b_cst = {
        "bqt": np.ascontiguousarray(
            np.asarray(inputs["bq"], np.float32).reshape(H, 64).T
        ),
        "bkt": np.ascontiguousarray(
            np.asarray(inputs["bk"], np.float32).reshape(H, 64).T
        ),
        "bv_b": np.asarray(inputs["bv"], np.float32).reshape(1, E).astype(bft),
        "bo_b": np.asarray(inputs["bo"], np.float32).reshape(1, E).astype(bft),
    }

    if _nc_cache is None:
        _nc_cache = build_bass()
    nc = _nc_cache

    in_maps = []
    for core in range(NCORES):
        b0 = core * BL
        x = query[:, b0 : b0 + BL, :]  # [T, BL, E]
        xr = np.ascontiguousarray(x.transpose(1, 0, 2).reshape(ROWS, E))
        xt = np.ascontiguousarray(
            xr.T.astype(bft).reshape(8, 128, ROWS).transpose(1, 0, 2)
        ).reshape(128, 8 * ROWS)
        in_maps.append(
            {
                "xt_t": xt,
                "kvt": kvt_all[b0 : b0 + BL],
                "maskt": np.ascontiguousarray(mm[b0 : b0 + BL]),
                **wt_tiles,
                **b_cst,
            }
        )

    res = run_bass_kernel_spmd(nc, in_maps, core_ids=list(range(NCORES)), trace=TRACE)
    LAST_RESULTS = res
    outs = []
    for core in range(NCORES):
        o = res.results[core]["out"].reshape(BL, T, E).transpose(1, 0, 2)
        outs.append(o)
    return np.concatenate(outs, axis=1).astype(np.float32)


# revision 30
# speedup vs baseline: 1.0338x; 1.0338x over previous
"""Trainium2 Bass kernel for AttentionForONNX decode-path self-attention.

Problem shapes (hardcoded): T=4, B=32, E=1024, H=16, HD=64, CACHE=4096, S=4100.
Sharding: batch B=32 split across 8 cores (BL=4 batches/core). Each core runs
the full attention for its 4 batches x 16 heads independently (no collectives);
host concatenates outputs on B.

Host-side prep (part of the sharding step): K cache is uploaded pre-transposed
and pre-tiled in bf16 (head-pairs interleaved on partitions), V cache bf16
pre-tiled to match the score chunk layout, weights uploaded as W^T bf16 tiles,
x as x^T bf16 tiles, and the key-padding mask as a pre-broadcast multiplicative
bf16 mask. This halves HBM traffic vs fp32 and removes all on-chip transposes
of the large operands.

Per-core kernel (memory-bound; K+V caches = 67MB/core dominate):
  - Q/K projections computed transposed (lhsT = W^T chunk, rhs = x^T chunk) so
    q^T/k_new^T land hd-major with no extra transposes; bias added via a fused
    DVE tensor_scalar_add on the PSUM->SBUF copy. V/out projections computed
    natural with a ones-row bias matmul.
  - Main loop over 32 (batch, head-pair) groups: one 1MB DMA for K^T of two
    heads [128=2*hd, 4096=s], one for V of two heads; 16 score matmuls
    (lhsT=K^T chunk [128,128], rhs=stacked q [128,8]) + 2 tail matmuls; one
    Exp ACT over [128, 264] (scale=0.125 folds the 1/sqrt(HD)); one DVE
    multiply applies the key-padding mask multiplicatively (masked prob = 0);
    64+2 PV matmuls accumulate O natural [4, 64] per head; a ones-column
    matmul gives Z; DVE reduce/reciprocal/transpose produce 1/Z per-partition
    and a tensor_scalar_mul writes normalized O.
"""

import numpy as np

T, B, E = 4, 32, 1024
H, HD = 16, 64
CACHE = 4096
S = CACHE + T
NCORES = 8
BL = B // NCORES  # batches per core = 4
ROWS = T * BL  # 16 projection rows per core, (b, t) order
NHP = H // 2  # 8 head-pairs
NGRP = BL * NHP  # 32 (b, head-pair) groups per core
NCH = CACHE // 128  # 32 s-chunks of 128
SCW = 8 * NCH + 8  # 264 score cols: (c, g, t) main + tail block


def build_bass():
    import concourse.bass as bass
    import concourse.bacc as bacc
    import concourse.mybir as mybir
    from concourse.masks import make_identity
    from concourse.tile import TileContext

    f32 = mybir.dt.float32
    bf16 = mybir.dt.bfloat16
    AF = mybir.ActivationFunctionType

    nc = bacc.Bacc(None)

    # ---- DRAM inputs (host pre-tiled, bf16 unless noted) ----
    xt_t = nc.dram_tensor("xt_t", [128, 8 * ROWS], bf16, kind="ExternalInput")
    kvt = nc.dram_tensor("kvt", [BL, NHP, 128, 8192], bf16, kind="ExternalInput")
    maskt = nc.dram_tensor("maskt", [BL, 128, SCW], bf16, kind="ExternalInput")
    wqt = nc.dram_tensor("wqt", [128, 8192], bf16, kind="ExternalInput")
    wkt = nc.dram_tensor("wkt", [128, 8192], bf16, kind="ExternalInput")
    wvt = nc.dram_tensor("wvt", [128, 8192], bf16, kind="ExternalInput")
    wot = nc.dram_tensor("wot", [128, 8192], bf16, kind="ExternalInput")
    bqt = nc.dram_tensor("bqt", [64, H], f32, kind="ExternalInput")
    bkt = nc.dram_tensor("bkt", [64, H], f32, kind="ExternalInput")
    bv_b = nc.dram_tensor("bv_b", [1, E], bf16, kind="ExternalInput")
    bo_b = nc.dram_tensor("bo_b", [1, E], bf16, kind="ExternalInput")
    out = nc.dram_tensor("out", [ROWS, E], f32, kind="ExternalOutput")

    with TileContext(nc) as tc:
        with (
            tc.tile_pool(name="const", bufs=1) as constp,
            tc.tile_pool(name="wts", bufs=1) as wtsp,
            tc.tile_pool(name="kv", bufs=3) as kvp,
            tc.tile_pool(name="ptp", bufs=2) as ptp,
            tc.tile_pool(name="ztp", bufs=2) as ztp,
            tc.tile_pool(name="ps_sc", bufs=2, space="PSUM") as ps_sc,
            tc.tile_pool(name="ps_pv", bufs=3, space="PSUM") as ps_pv,
            tc.tile_pool(name="ps_pj1", bufs=1, space="PSUM") as ps_pj1,
            tc.tile_pool(name="ps_pj2", bufs=1, space="PSUM") as ps_pj2,
        ):
            # ---- startup loads, ordered so q-projection can start ASAP ----
            wq_sb = wtsp.tile([128, 8192], bf16, tag="wq")
            nc.sync.dma_start(out=wq_sb[:, :], in_=wqt[:, :])
            xt = constp.tile([128, 8 * ROWS], bf16, tag="xt")
            nc.sync.dma_start(out=xt[:, :], in_=xt_t[:, :])
            bq_sb = constp.tile([64, H], f32, tag="bq")
            nc.sync.dma_start(out=bq_sb[:, :], in_=bqt[:, :])
            wk_sb = wtsp.tile([128, 8192], bf16, tag="wk")
            nc.sync.dma_start(out=wk_sb[:, :], in_=wkt[:, :])
            bk_sb = constp.tile([64, H], f32, tag="bk")
            nc.sync.dma_start(out=bk_sb[:, :], in_=bkt[:, :])
            wv_sb = wtsp.tile([128, 8192], bf16, tag="wv")
            nc.sync.dma_start(out=wv_sb[:, :], in_=wvt[:, :])
            bv_sb = constp.tile([1, E], bf16, tag="bv")
            nc.sync.dma_start(out=bv_sb[:, :], in_=bv_b[:, :])
            mask_sb = constp.tile([128, BL * SCW], bf16, tag="mask")
            for b in range(BL):
                nc.sync.dma_start(
                    out=mask_sb[:, SCW * b : SCW * (b + 1)], in_=maskt[b]
                )
            bo_sb = constp.tile([1, E], bf16, tag="bo")
            nc.sync.dma_start(out=bo_sb[:, :], in_=bo_b[:, :])
            wo_sb = wtsp.tile([128, 8192], bf16, tag="wo")
            nc.sync.dma_start(out=wo_sb[:, :], in_=wot[:, :])

            # ---- constants ----
            ones_col = constp.tile([128, 1], bf16, tag="ones_col")
            nc.vector.memset(ones_col[:, :], 1.0)
            ones_row = constp.tile([1, ROWS], bf16, tag="ones_row")
            nc.vector.memset(ones_row[:, :], 1.0)
            ones_r64 = constp.tile([1, 64], f32, tag="ones_r64")
            nc.vector.memset(ones_r64[:, :], 1.0)

            # ---- transposed q/k projections: pT[64, 16(h)*16(b,t)] ----
            # wq_sb layout: [:, (c*16 + h)*64 : +64] = W^T rows e-chunk c, cols
            # j in [64h, 64h+64).  psum [64, 16] per h accumulated over c.
            def projT(w_sb, bias_sb, dest):
                pj = ps_pj1.tile([128, 16 * H], f32, tag="pj1")
                for h in range(H):
                    for c in range(8):
                        nc.tensor.matmul(
                            pj[0:64, 16 * h : 16 * (h + 1)],
                            w_sb[:, (c * 16 + h) * 64 : (c * 16 + h) * 64 + 64],
                            xt[:, ROWS * c : ROWS * (c + 1)],
                            start=(c == 0),
                            stop=(c == 7),
                        )
                for h in range(H):
                    nc.vector.tensor_scalar_add(
                        dest[0:64, 16 * h : 16 * (h + 1)],
                        pj[0:64, 16 * h : 16 * (h + 1)],
                        bias_sb[0:64, h : h + 1],
                    )

            qT = constp.tile([64, 16 * H], bf16, tag="qT")
            projT(wq_sb, bq_sb, qT)
            kT = constp.tile([64, 16 * H], bf16, tag="kT")
            projT(wk_sb, bk_sb, kT)

            # q duplicated on partitions 64:128 (SBUF->SBUF DMA partition move)
            qdup = constp.tile([128, 16 * H], bf16, tag="qdup")
            nc.sync.dma_start(out=qdup[64:128, :], in_=qT[0:64, :])

            # q2_stack [128, 8*NGRP]: group g=(b*NHP+hp): rows 0:64 cols 8g+0:4
            # = q^T(b, 2hp); rows 64:128 cols 8g+4:8 = q^T(b, 2hp+1)
            q2s = constp.tile([128, 8 * NGRP], bf16, tag="q2s")
            nc.vector.memset(q2s[:, :], 0.0)
            q2s_top = q2s[0:64, :].rearrange("p (b r) -> p b r", r=8 * NHP)
            q2s_bot = q2s[64:128, :].rearrange("p (b r) -> p b r", r=8 * NHP)
            for hp in range(NHP):
                # src cols for head h: 16h + 4b + t ; dst cols 8*(b*8+hp)+...
                nc.vector.tensor_copy(
                    q2s_top[:, :, 8 * hp : 8 * hp + 4],
                    qT[0:64, 16 * (2 * hp) : 16 * (2 * hp) + 16]
                    .rearrange("p (b t) -> p b t", t=T),
                )
                nc.vector.tensor_copy(
                    q2s_bot[:, :, 8 * hp + 4 : 8 * hp + 8],
                    qdup[64:128, 16 * (2 * hp + 1) : 16 * (2 * hp + 1) + 16]
                    .rearrange("p (b t) -> p b t", t=T),
                )

            # knt2p [64, H*128]: head h block cols 128h:128h+128, cols 0:16 =
            # k_new^T (b', t'), rest zero (pads tail-score out to 128 rows)
            knt2p = constp.tile([64, H * 128], bf16, tag="knt2p")
            nc.vector.memset(knt2p[:, :], 0.0)
            for h in range(H):
                nc.vector.tensor_copy(
                    knt2p[0:64, 128 * h : 128 * h + 16],
                    kT[0:64, 16 * h : 16 * (h + 1)],
                )

            # ---- natural v projection: vn [16, 1024] bf16 ----
            pj2 = ps_pj2.tile([ROWS, E], f32, tag="pj2")
            for half in range(2):
                sl = slice(512 * half, 512 * (half + 1))
                for c in range(8):
                    nc.tensor.matmul(
                        pj2[:, sl],
                        xt[:, ROWS * c : ROWS * (c + 1)],
                        wv_sb[:, 1024 * c + 512 * half : 1024 * c + 512 * (half + 1)],
                        start=(c == 0),
                        stop=False,
                    )
                nc.tensor.matmul(
                    pj2[:, sl], ones_row[:, :], bv_sb[:, sl], start=False, stop=True
                )
            vn = constp.tile([ROWS, E], bf16, tag="vn")
            nc.vector.tensor_copy(vn[:, :], pj2[:, :])

            # ---- O^T accumulator (unnormalized, f32):
            #      rows (h%2)*64+hd, cols ROWS*hp + 4b + t ----
            otu = constp.tile([128, 8 * ROWS], f32, tag="otu")
            # 1/Z for all groups: [0:1, 16g : 16g+8] = 1/z (gg, t)
            zall = constp.tile([1, 16 * NGRP], f32, tag="zall")

            def stage_a(b, hp, pt, v2, pv):
                # PV (transposed output: O^T via lhsT=v chunk) + Z matmuls +
                # Z reduce/reciprocal; emitted one iteration late so these PE
                # matmuls fill the exp/mask bubble after the next group's
                # score matmuls.
                for gg in range(2):
                    for c in range(NCH):
                        nc.tensor.matmul(
                            pv[64 * gg : 64 * (gg + 1), 0:T],
                            v2[:, 2048 * gg + 64 * c : 2048 * gg + 64 * (c + 1)],
                            pt[:, 8 * c + 4 * gg : 8 * c + 4 * (gg + 1)],
                            start=(c == 0),
                            stop=False,
                        )
                    h = 2 * hp + gg
                    nc.tensor.matmul(
                        pv[64 * gg : 64 * (gg + 1), 0:T],
                        vn[:, 64 * h : 64 * (h + 1)],
                        pt[0:ROWS, 8 * NCH + 4 * gg : 8 * NCH + 4 * (gg + 1)],
                        start=False,
                        stop=True,
                    )
                # Z row-sums via ones-column matmul -> [1, 264]
                nc.tensor.matmul(
                    pv[0:1, 128 : 128 + SCW],
                    ones_col[:, :],
                    pt[:, :],
                    start=True,
                    stop=True,
                )
                g = b * NHP + hp
                zs = ztp.tile([1, 16 * NGRP], f32, tag="zsum")
                nc.vector.reduce_sum(
                    zs[0:1, 16 * g + 8 : 16 * g + 16],
                    pv[0:1, 128 : 128 + SCW].rearrange("p (c x) -> p x c", x=8),
                    axis=mybir.AxisListType.X,
                )
                nc.vector.reciprocal(
                    zall[0:1, 16 * g : 16 * g + 8], zs[0:1, 16 * g + 8 : 16 * g + 16]
                )
                # park the unnormalized O^T block (Scalar, off the DVE queue)
                nc.scalar.activation(
                    otu[:, ROWS * hp + T * b : ROWS * hp + T * (b + 1)],
                    pv[:, 0:T],
                    AF.Copy,
                )

            # ---- main attention loop over 32 groups (software-pipelined) ----
            pend_a = None
            for b in range(BL):
                for hp in range(NHP):
                    g = b * NHP + hp
                    kv2 = kvp.tile([128, 8192], bf16, tag="kv2")
                    nc.sync.dma_start(out=kv2[:, :], in_=kvt[b, hp])
                    kt2 = kv2[:, 0:CACHE]
                    v2 = kv2[:, CACHE:8192]

                    sc = ps_sc.tile([128, SCW], f32, tag="sc")
                    # main scores: S^T[s=128c+p, (g,t)] for both heads
                    for c in range(NCH):
                        nc.tensor.matmul(
                            sc[:, 8 * c : 8 * (c + 1)],
                            kt2[:, 128 * c : 128 * (c + 1)],
                            q2s[:, 8 * g : 8 * (g + 1)],
                            start=True,
                            stop=True,
                        )
                    # tail scores: rows (b', t'), own-b rows kept by the mask
                    for gg in range(2):
                        h = 2 * hp + gg
                        nc.tensor.matmul(
                            sc[:, 8 * NCH + 4 * gg : 8 * NCH + 4 * (gg + 1)],
                            knt2p[:, 128 * h : 128 * (h + 1)],
                            qT[0:64, 16 * h + 4 * b : 16 * h + 4 * b + 4],
                            start=True,
                            stop=True,
                        )

                    # P = exp(S/8) * mask, emitted BEFORE the delayed stages
                    # so the mask multiply is never queued behind the (PE-
                    # blocked) Z-chain ops in the strict-FIFO DVE queue.
                    pt_raw = ptp.tile([128, SCW], bf16, tag="pt_raw")
                    nc.scalar.activation(pt_raw[:, :], sc[:, :], AF.Exp, scale=0.125)
                    pt = ptp.tile([128, SCW], bf16, tag="pt")
                    nc.vector.tensor_mul(
                        pt[:, :], pt_raw[:, :], mask_sb[:, SCW * b : SCW * (b + 1)]
                    )
                    pv = ps_pv.tile([128, 512], f32, tag="pv")

                    if pend_a is not None:
                        stage_a(*pend_a)
                    pend_a = (b, hp, pt, v2, pv)
            stage_a(*pend_a)

            # ---- batched normalization: O^T * (1/Z broadcast) ----
            zz = ps_pj1.tile([128, 16 * H], f32, tag="pj1")
            for g in range(NGRP):
                b, hp = divmod(g, NHP)
                col = ROWS * hp + T * b
                for gg in range(2):
                    nc.tensor.matmul(
                        zz[64 * gg : 64 * (gg + 1), col : col + T],
                        ones_r64[:, :],
                        zall[0:1, 16 * g + 4 * gg : 16 * g + 4 * gg + 4],
                        start=True,
                        stop=True,
                    )
            zzs = constp.tile([128, 8 * ROWS], f32, tag="zzs")
            nc.scalar.activation(zzs[:, :], zz[:, 0 : 8 * ROWS], AF.Copy)
            ot = constp.tile([128, 8 * ROWS], bf16, tag="ot")
            nc.vector.tensor_mul(ot[:, :], otu[:, :], zzs[:, :])

            # ---- out projection ----
            out_ps = ps_pj2.tile([ROWS, E], f32, tag="pj2")
            for half in range(2):
                sl = slice(512 * half, 512 * (half + 1))
                for c in range(8):
                    nc.tensor.matmul(
                        out_ps[:, sl],
                        ot[:, ROWS * c : ROWS * (c + 1)],
                        wo_sb[:, 1024 * c + 512 * half : 1024 * c + 512 * (half + 1)],
                        start=(c == 0),
                        stop=False,
                    )
                nc.tensor.matmul(
                    out_ps[:, sl], ones_row[:, :], bo_sb[:, sl], start=False, stop=True
                )
            out_sb = constp.tile([ROWS, E], f32, tag="outsb")
            nc.vector.tensor_copy(out_sb[:, :], out_ps[:, :])
            nc.sync.dma_start(out=out[:, :], in_=out_sb[:, :])

    nc.finalize()
    return nc


_nc_cache = None
TRACE = False
LAST_RESULTS = None


def kernel(**inputs):
    global _nc_cache, LAST_RESULTS
    from concourse.bass_utils import run_bass_kernel_spmd
    import ml_dtypes

    bft = ml_dtypes.bfloat16

    query = np.asarray(inputs["query"], dtype=np.float32)
    mask = np.asarray(inputs["key_padding_mask"])
    kc = np.asarray(inputs["self_p_k"], dtype=np.float32)
    vc = np.asarray(inputs["self_p_v"], dtype=np.float32)

    # one packed [128, 8192] tile per (b, head-pair): cols 0:4096 = K^T with
    # the two heads stacked on partitions, cols 4096:8192 = V pre-tiled so
    # head g chunk c (cols 64c:64c+64) holds v rows s=128c+p
    kvt_all = np.empty((B, NHP, 128, 8192), dtype=bft)
    kvt_all[:, :, :, :CACHE] = (
        kc.astype(bft).reshape(B, NHP, 2, CACHE, HD).transpose(0, 1, 2, 4, 3)
    ).reshape(B, NHP, 128, CACHE)
    kvt_all[:, :, :, CACHE:] = (
        vc.astype(bft).reshape(B, NHP, 2, NCH, 128, HD).transpose(0, 1, 4, 2, 3, 5)
    ).reshape(B, NHP, 128, 2 * 2048)

    # multiplicative mask, pre-broadcast to the score layout [B, 128, SCW]
    minv = (~mask).astype(np.float32)  # [B, S]: 1 keep, 0 drop
    mm = np.zeros((B, 128, SCW), dtype=np.float32)
    main = minv[:, :CACHE].reshape(B, NCH, 128).transpose(0, 2, 1)  # [B, 128, c]
    mm[:, :, : 8 * NCH] = np.repeat(main, 8, axis=2)
    tail = minv[:, CACHE:]  # [B, T]
    for b in range(B):
        bl = b % BL  # local batch index on its core
        for j in range(T):
            for gg in range(2):
                for t in range(T):
                    mm[b, 4 * bl + j, 8 * NCH + 4 * gg + t] = tail[b, j]
    mm = mm.astype(bft)

    def wT_tiles_T(w):  # for transposed projections (lhsT layout)
        wt = w.astype(bft).T  # [e, j]
        return np.ascontiguousarray(
            wt.reshape(8, 128, H, 64).transpose(1, 0, 2, 3).reshape(128, 8192)
        )

    def wT_tiles_N(w):  # for natural projections (rhs layout)
        wt = w.astype(bft).T  # [e, j]
        return np.ascontiguousarray(
            wt.reshape(8, 128, E).transpose(1, 0, 2).reshape(128, 8192)
        )

    # note: the 1/sqrt(HD) q-scaling is folded into the on-chip exp scale
    wt_tiles = {
        "wqt": wT_tiles_T(np.asarray(inputs["Wq"], np.float32)),
        "wkt": wT_tiles_T(np.asarray(inputs["Wk"], np.float32)),
        "wvt": wT_tiles_N(np.asarray(inputs["Wv"], np.float32)),
        "wot": wT_tiles_N(np.asarray(inputs["Wo"], np.float32)),
    }
    b_cst = {
        "bqt": np.ascontiguousarray(
            np.asarray(inputs["bq"], np.float32).reshape(H, 64).T
        ),
        "bkt": np.ascontiguousarray(
            np.asarray(inputs["bk"], np.float32).reshape(H, 64).T
        ),
        "bv_b": np.asarray(inputs["bv"], np.float32).reshape(1, E).astype(bft),
        "bo_b": np.asarray(inputs["bo"], np.float32).reshape(1, E).astype(bft),
    }

    if _nc_cache is None:
        _nc_cache = build_bass()
    nc = _nc_cache

    in_maps = []
    for core in range(NCORES):
        b0 = core * BL
        x = query[:, b0 : b0 + BL, :]  # [T, BL, E]
        xr = np.ascontiguousarray(x.transpose(1, 0, 2).reshape(ROWS, E))
        xt = np.ascontiguousarray(
            xr.T.astype(bft).reshape(8, 128, ROWS).transpose(1, 0, 2)
        ).reshape(128, 8 * ROWS)
        in_maps.append(
            {
                "xt_t": xt,
                "kvt": kvt_all[b0 : b0 + BL],
                "maskt": np.ascontiguousarray(mm[b0 : b0 + BL]),
                **wt_tiles,
                **b_cst,
            }
        )

    res = run_bass_kernel_spmd(nc, in_maps, core_ids=list(range(NCORES)), trace=TRACE)
    LAST_RESULTS = res
    outs = []
    for core in range(NCORES):
        o = res.results[core]["out"].reshape(BL, T, E).transpose(1, 0, 2)
        outs.append(o)
    return np.concatenate(outs, axis=1).astype(np.float32)


# revision 33
# speedup vs baseline: 1.2594x; 1.2182x over previous
"""Trainium2 Bass kernel for AttentionForONNX decode-path self-attention.

Problem shapes (hardcoded): T=4, B=32, E=1024, H=16, HD=64, CACHE=4096, S=4100.
Sharding: batch B=32 split across 8 cores (BL=4 batches/core). Each core runs
the full attention for its 4 batches x 16 heads independently (no collectives);
host concatenates outputs on B.

Host-side prep (part of the sharding step): K cache is uploaded pre-transposed
and pre-tiled in bf16 (head-pairs interleaved on partitions), V cache bf16
pre-tiled to match the score chunk layout, weights uploaded as W^T bf16 tiles,
x as x^T bf16 tiles, and the key-padding mask as a pre-broadcast multiplicative
bf16 mask. This halves HBM traffic vs fp32 and removes all on-chip transposes
of the large operands.

Per-core kernel (memory-bound; K+V caches = 67MB/core dominate):
  - Q/K projections computed transposed (lhsT = W^T chunk, rhs = x^T chunk) so
    q^T/k_new^T land hd-major with no extra transposes; bias added via a fused
    DVE tensor_scalar_add on the PSUM->SBUF copy. V/out projections computed
    natural with a ones-row bias matmul.
  - Main loop over 32 (batch, head-pair) groups: one 1MB DMA for K^T of two
    heads [128=2*hd, 4096=s], one for V of two heads; 16 score matmuls
    (lhsT=K^T chunk [128,128], rhs=stacked q [128,8]) + 2 tail matmuls; one
    Exp ACT over [128, 264] (scale=0.125 folds the 1/sqrt(HD)); one DVE
    multiply applies the key-padding mask multiplicatively (masked prob = 0);
    64+2 PV matmuls accumulate O natural [4, 64] per head; a ones-column
    matmul gives Z; DVE reduce/reciprocal/transpose produce 1/Z per-partition
    and a tensor_scalar_mul writes normalized O.
"""

import numpy as np

T, B, E = 4, 32, 1024
H, HD = 16, 64
CACHE = 4096
S = CACHE + T
NCORES = 8
BL = B // NCORES  # batches per core = 4
ROWS = T * BL  # 16 projection rows per core, (b, t) order
NHP = H // 2  # 8 head-pairs
NGRP = BL * NHP  # 32 (b, head-pair) groups per core
NCH = CACHE // 128  # 32 s-chunks of 128
SCW = 8 * NCH + 8  # 264 score cols: (c, g, t) main + tail block


def build_bass():
    import concourse.bass as bass
    import concourse.bacc as bacc
    import concourse.mybir as mybir
    from concourse.masks import make_identity
    from concourse.tile import TileContext

    f32 = mybir.dt.float32
    bf16 = mybir.dt.bfloat16
    AF = mybir.ActivationFunctionType

    nc = bacc.Bacc(None)

    # ---- DRAM inputs (host pre-tiled, bf16 unless noted) ----
    xt_t = nc.dram_tensor("xt_t", [128, 8 * ROWS], bf16, kind="ExternalInput")
    kvt = nc.dram_tensor("kvt", [BL, NHP, 128, 8192], bf16, kind="ExternalInput")
    maskt = nc.dram_tensor("maskt", [BL, 128, SCW], bf16, kind="ExternalInput")
    wqt = nc.dram_tensor("wqt", [128, 8192], bf16, kind="ExternalInput")
    wkt = nc.dram_tensor("wkt", [128, 8192], bf16, kind="ExternalInput")
    wvt = nc.dram_tensor("wvt", [128, 8192], bf16, kind="ExternalInput")
    wot = nc.dram_tensor("wot", [128, 8192], bf16, kind="ExternalInput")
    bqt = nc.dram_tensor("bqt", [64, H], f32, kind="ExternalInput")
    bkt = nc.dram_tensor("bkt", [64, H], f32, kind="ExternalInput")
    bv_b = nc.dram_tensor("bv_b", [1, E], bf16, kind="ExternalInput")
    bo_b = nc.dram_tensor("bo_b", [1, E], bf16, kind="ExternalInput")
    out = nc.dram_tensor("out", [ROWS, E], f32, kind="ExternalOutput")

    with TileContext(nc) as tc:
        with (
            tc.tile_pool(name="const", bufs=1) as constp,
            tc.tile_pool(name="wts", bufs=1) as wtsp,
            tc.tile_pool(name="kv", bufs=3) as kvp,
            tc.tile_pool(name="ptp", bufs=2) as ptp,
            tc.tile_pool(name="ztp", bufs=2) as ztp,
            tc.tile_pool(name="ps_sc", bufs=2, space="PSUM") as ps_sc,
            tc.tile_pool(name="ps_pv", bufs=3, space="PSUM") as ps_pv,
            tc.tile_pool(name="ps_pj1", bufs=1, space="PSUM") as ps_pj1,
            tc.tile_pool(name="ps_pj2", bufs=1, space="PSUM") as ps_pj2,
        ):
            # ---- startup loads, ordered so q-projection can start ASAP ----
            wq_sb = wtsp.tile([128, 8192], bf16, tag="wq")
            nc.sync.dma_start(out=wq_sb[:, :], in_=wqt[:, :])
            xt = constp.tile([128, 8 * ROWS], bf16, tag="xt")
            nc.sync.dma_start(out=xt[:, :], in_=xt_t[:, :])
            bq_sb = constp.tile([64, H], f32, tag="bq")
            nc.sync.dma_start(out=bq_sb[:, :], in_=bqt[:, :])
            wk_sb = wtsp.tile([128, 8192], bf16, tag="wk")
            nc.sync.dma_start(out=wk_sb[:, :], in_=wkt[:, :])
            bk_sb = constp.tile([64, H], f32, tag="bk")
            nc.sync.dma_start(out=bk_sb[:, :], in_=bkt[:, :])
            wv_sb = wtsp.tile([128, 8192], bf16, tag="wv")
            nc.sync.dma_start(out=wv_sb[:, :], in_=wvt[:, :])
            bv_sb = constp.tile([1, E], bf16, tag="bv")
            nc.sync.dma_start(out=bv_sb[:, :], in_=bv_b[:, :])
            mask_sb = constp.tile([128, BL * SCW], bf16, tag="mask")
            for b in range(BL):
                nc.sync.dma_start(
                    out=mask_sb[:, SCW * b : SCW * (b + 1)], in_=maskt[b]
                )
            bo_sb = constp.tile([1, E], bf16, tag="bo")
            nc.sync.dma_start(out=bo_sb[:, :], in_=bo_b[:, :])
            wo_sb = wtsp.tile([128, 8192], bf16, tag="wo")
            nc.sync.dma_start(out=wo_sb[:, :], in_=wot[:, :])

            # ---- constants ----
            ones_col = constp.tile([128, 1], bf16, tag="ones_col")
            nc.vector.memset(ones_col[:, :], 1.0)
            ones_row = constp.tile([1, ROWS], bf16, tag="ones_row")
            nc.vector.memset(ones_row[:, :], 1.0)
            ones_r64 = constp.tile([1, 64], f32, tag="ones_r64")
            nc.vector.memset(ones_r64[:, :], 1.0)

            # ---- transposed q/k projections: pT[64, 16(h)*16(b,t)] ----
            # wq_sb layout: [:, (c*16 + h)*64 : +64] = W^T rows e-chunk c, cols
            # j in [64h, 64h+64).  psum [64, 16] per h accumulated over c.
            def projT(w_sb, bias_sb, dest):
                pj = ps_pj1.tile([128, 16 * H], f32, tag="pj1")
                for h in range(H):
                    for c in range(8):
                        nc.tensor.matmul(
                            pj[0:64, 16 * h : 16 * (h + 1)],
                            w_sb[:, (c * 16 + h) * 64 : (c * 16 + h) * 64 + 64],
                            xt[:, ROWS * c : ROWS * (c + 1)],
                            start=(c == 0),
                            stop=(c == 7),
                        )
                for h in range(H):
                    nc.vector.tensor_scalar_add(
                        dest[0:64, 16 * h : 16 * (h + 1)],
                        pj[0:64, 16 * h : 16 * (h + 1)],
                        bias_sb[0:64, h : h + 1],
                    )

            qT = constp.tile([64, 16 * H], bf16, tag="qT")
            projT(wq_sb, bq_sb, qT)
            kT = constp.tile([64, 16 * H], bf16, tag="kT")
            projT(wk_sb, bk_sb, kT)

            # q duplicated on partitions 64:128 (SBUF->SBUF DMA partition move)
            qdup = constp.tile([128, 16 * H], bf16, tag="qdup")
            nc.sync.dma_start(out=qdup[64:128, :], in_=qT[0:64, :])

            # q2_stack [128, 8*NGRP]: group g=(b*NHP+hp): rows 0:64 cols 8g+0:4
            # = q^T(b, 2hp); rows 64:128 cols 8g+4:8 = q^T(b, 2hp+1)
            q2s = constp.tile([128, 8 * NGRP], bf16, tag="q2s")
            nc.vector.memset(q2s[:, :], 0.0)
            q2s_top = q2s[0:64, :].rearrange("p (b r) -> p b r", r=8 * NHP)
            q2s_bot = q2s[64:128, :].rearrange("p (b r) -> p b r", r=8 * NHP)
            for hp in range(NHP):
                # src cols for head h: 16h + 4b + t ; dst cols 8*(b*8+hp)+...
                nc.vector.tensor_copy(
                    q2s_top[:, :, 8 * hp : 8 * hp + 4],
                    qT[0:64, 16 * (2 * hp) : 16 * (2 * hp) + 16]
                    .rearrange("p (b t) -> p b t", t=T),
                )
                nc.vector.tensor_copy(
                    q2s_bot[:, :, 8 * hp + 4 : 8 * hp + 8],
                    qdup[64:128, 16 * (2 * hp + 1) : 16 * (2 * hp + 1) + 16]
                    .rearrange("p (b t) -> p b t", t=T),
                )

            # knt2p [64, H*128]: head h block cols 128h:128h+128, cols 0:16 =
            # k_new^T (b', t'), rest zero (pads tail-score out to 128 rows)
            knt2p = constp.tile([64, H * 128], bf16, tag="knt2p")
            nc.vector.memset(knt2p[:, :], 0.0)
            for h in range(H):
                nc.vector.tensor_copy(
                    knt2p[0:64, 128 * h : 128 * h + 16],
                    kT[0:64, 16 * h : 16 * (h + 1)],
                )

            # ---- natural v projection: vn [16, 1024] bf16 ----
            pj2 = ps_pj2.tile([ROWS, E], f32, tag="pj2")
            for half in range(2):
                sl = slice(512 * half, 512 * (half + 1))
                for c in range(8):
                    nc.tensor.matmul(
                        pj2[:, sl],
                        xt[:, ROWS * c : ROWS * (c + 1)],
                        wv_sb[:, 1024 * c + 512 * half : 1024 * c + 512 * (half + 1)],
                        start=(c == 0),
                        stop=False,
                    )
                nc.tensor.matmul(
                    pj2[:, sl], ones_row[:, :], bv_sb[:, sl], start=False, stop=True
                )
            vn = constp.tile([ROWS, E], bf16, tag="vn")
            nc.vector.tensor_copy(vn[:, :], pj2[:, :])

            # ---- O^T accumulator (unnormalized, f32):
            #      rows (h%2)*64+hd, cols ROWS*hp + 4b + t ----
            otu = constp.tile([128, 8 * ROWS], f32, tag="otu")
            # 1/Z for all groups: [0:1, 16g : 16g+8] = 1/z (gg, t)
            zall = constp.tile([1, 16 * NGRP], f32, tag="zall")

            def stage_a(b, hp, pt, v2, pv):
                # PV with both heads in one [128,128] stationary: out[0:64,
                # 0:4] accumulates O^T(h0), out[64:128, 4:8] O^T(h1); the
                # complementary blocks accumulate cross-head junk that is
                # never read.  Emitted one iteration late so these PE matmuls
                # fill the exp/mask bubble after the next group's scores.
                for c in range(NCH):
                    nc.tensor.matmul(
                        pv[:, 0:8],
                        v2[:, 128 * c : 128 * (c + 1)],
                        pt[:, 8 * c : 8 * (c + 1)],
                        start=(c == 0),
                        stop=False,
                    )
                for gg in range(2):
                    h = 2 * hp + gg
                    nc.tensor.matmul(
                        pv[64 * gg : 64 * (gg + 1), 4 * gg : 4 * (gg + 1)],
                        vn[:, 64 * h : 64 * (h + 1)],
                        pt[0:ROWS, 8 * NCH + 4 * gg : 8 * NCH + 4 * (gg + 1)],
                        start=False,
                        stop=(gg == 1),
                    )
                # Z row-sums via ones-column matmul -> [1, 264]
                nc.tensor.matmul(
                    pv[0:1, 128 : 128 + SCW],
                    ones_col[:, :],
                    pt[:, :],
                    start=True,
                    stop=True,
                )
                g = b * NHP + hp
                zs = ztp.tile([1, 16 * NGRP], f32, tag="zsum")
                nc.vector.reduce_sum(
                    zs[0:1, 16 * g + 8 : 16 * g + 16],
                    pv[0:1, 128 : 128 + SCW].rearrange("p (c x) -> p x c", x=8),
                    axis=mybir.AxisListType.X,
                )
                nc.vector.reciprocal(
                    zall[0:1, 16 * g : 16 * g + 8], zs[0:1, 16 * g + 8 : 16 * g + 16]
                )
                # park the unnormalized O^T blocks (Scalar, off the DVE queue)
                col = ROWS * hp + T * b
                nc.scalar.activation(
                    otu[0:64, col : col + T], pv[0:64, 0:T], AF.Copy
                )
                nc.scalar.activation(
                    otu[64:128, col : col + T], pv[64:128, T : 2 * T], AF.Copy
                )

            # ---- main attention loop over 32 groups (software-pipelined) ----
            pend_a = None
            for b in range(BL):
                for hp in range(NHP):
                    g = b * NHP + hp
                    kv2 = kvp.tile([128, 8192], bf16, tag="kv2")
                    nc.sync.dma_start(out=kv2[:, :], in_=kvt[b, hp])
                    kt2 = kv2[:, 0:CACHE]
                    v2 = kv2[:, CACHE:8192]

                    sc = ps_sc.tile([128, SCW], f32, tag="sc")
                    # main scores: S^T[s=128c+p, (g,t)] for both heads
                    for c in range(NCH):
                        nc.tensor.matmul(
                            sc[:, 8 * c : 8 * (c + 1)],
                            kt2[:, 128 * c : 128 * (c + 1)],
                            q2s[:, 8 * g : 8 * (g + 1)],
                            start=True,
                            stop=True,
                        )
                    # tail scores: rows (b', t'), own-b rows kept by the mask
                    for gg in range(2):
                        h = 2 * hp + gg
                        nc.tensor.matmul(
                            sc[:, 8 * NCH + 4 * gg : 8 * NCH + 4 * (gg + 1)],
                            knt2p[:, 128 * h : 128 * (h + 1)],
                            qT[0:64, 16 * h + 4 * b : 16 * h + 4 * b + 4],
                            start=True,
                            stop=True,
                        )

                    # P = exp(S/8) * mask, emitted BEFORE the delayed stages
                    # so the mask multiply is never queued behind the (PE-
                    # blocked) Z-chain ops in the strict-FIFO DVE queue.
                    pt_raw = ptp.tile([128, SCW], bf16, tag="pt_raw")
                    nc.scalar.activation(pt_raw[:, :], sc[:, :], AF.Exp, scale=0.125)
                    pt = ptp.tile([128, SCW], bf16, tag="pt")
                    nc.vector.tensor_mul(
                        pt[:, :], pt_raw[:, :], mask_sb[:, SCW * b : SCW * (b + 1)]
                    )
                    pv = ps_pv.tile([128, 512], f32, tag="pv")

                    if pend_a is not None:
                        stage_a(*pend_a)
                    pend_a = (b, hp, pt, v2, pv)
            stage_a(*pend_a)

            # ---- batched normalization: O^T * (1/Z broadcast) ----
            zz = ps_pj1.tile([128, 16 * H], f32, tag="pj1")
            for g in range(NGRP):
                b, hp = divmod(g, NHP)
                col = ROWS * hp + T * b
                for gg in range(2):
                    nc.tensor.matmul(
                        zz[64 * gg : 64 * (gg + 1), col : col + T],
                        ones_r64[:, :],
                        zall[0:1, 16 * g + 4 * gg : 16 * g + 4 * gg + 4],
                        start=True,
                        stop=True,
                    )
            zzs = constp.tile([128, 8 * ROWS], f32, tag="zzs")
            nc.scalar.activation(zzs[:, :], zz[:, 0 : 8 * ROWS], AF.Copy)
            ot = constp.tile([128, 8 * ROWS], bf16, tag="ot")
            nc.vector.tensor_mul(ot[:, :], otu[:, :], zzs[:, :])

            # ---- out projection ----
            out_ps = ps_pj2.tile([ROWS, E], f32, tag="pj2")
            for half in range(2):
                sl = slice(512 * half, 512 * (half + 1))
                for c in range(8):
                    nc.tensor.matmul(
                        out_ps[:, sl],
                        ot[:, ROWS * c : ROWS * (c + 1)],
                        wo_sb[:, 1024 * c + 512 * half : 1024 * c + 512 * (half + 1)],
                        start=(c == 0),
                        stop=False,
                    )
                nc.tensor.matmul(
                    out_ps[:, sl], ones_row[:, :], bo_sb[:, sl], start=False, stop=True
                )
            out_sb = constp.tile([ROWS, E], f32, tag="outsb")
            nc.vector.tensor_copy(out_sb[:, :], out_ps[:, :])
            nc.sync.dma_start(out=out[:, :], in_=out_sb[:, :])

    nc.finalize()
    return nc


_nc_cache = None
TRACE = False
LAST_RESULTS = None


def kernel(**inputs):
    global _nc_cache, LAST_RESULTS
    from concourse.bass_utils import run_bass_kernel_spmd
    import ml_dtypes

    bft = ml_dtypes.bfloat16

    query = np.asarray(inputs["query"], dtype=np.float32)
    mask = np.asarray(inputs["key_padding_mask"])
    kc = np.asarray(inputs["self_p_k"], dtype=np.float32)
    vc = np.asarray(inputs["self_p_v"], dtype=np.float32)

    # one packed [128, 8192] tile per (b, head-pair): cols 0:4096 = K^T with
    # the two heads stacked on partitions, cols 4096:8192 = V pre-tiled so
    # head g chunk c (cols 64c:64c+64) holds v rows s=128c+p
    kvt_all = np.empty((B, NHP, 128, 8192), dtype=bft)
    kvt_all[:, :, :, :CACHE] = (
        kc.astype(bft).reshape(B, NHP, 2, CACHE, HD).transpose(0, 1, 2, 4, 3)
    ).reshape(B, NHP, 128, CACHE)
    kvt_all[:, :, :, CACHE:] = (
        vc.astype(bft).reshape(B, NHP, 2, NCH, 128, HD).transpose(0, 1, 4, 3, 2, 5)
    ).reshape(B, NHP, 128, 2 * 2048)

    # multiplicative mask, pre-broadcast to the score layout [B, 128, SCW]
    minv = (~mask).astype(np.float32)  # [B, S]: 1 keep, 0 drop
    mm = np.zeros((B, 128, SCW), dtype=np.float32)
    main = minv[:, :CACHE].reshape(B, NCH, 128).transpose(0, 2, 1)  # [B, 128, c]
    mm[:, :, : 8 * NCH] = np.repeat(main, 8, axis=2)
    tail = minv[:, CACHE:]  # [B, T]
    for b in range(B):
        bl = b % BL  # local batch index on its core
        for j in range(T):
            for gg in range(2):
                for t in range(T):
                    mm[b, 4 * bl + j, 8 * NCH + 4 * gg + t] = tail[b, j]
    mm = mm.astype(bft)

    def wT_tiles_T(w):  # for transposed projections (lhsT layout)
        wt = w.astype(bft).T  # [e, j]
        return np.ascontiguousarray(
            wt.reshape(8, 128, H, 64).transpose(1, 0, 2, 3).reshape(128, 8192)
        )

    def wT_tiles_N(w):  # for natural projections (rhs layout)
        wt = w.astype(bft).T  # [e, j]
        return np.ascontiguousarray(
            wt.reshape(8, 128, E).transpose(1, 0, 2).reshape(128, 8192)
        )

    # note: the 1/sqrt(HD) q-scaling is folded into the on-chip exp scale
    wt_tiles = {
        "wqt": wT_tiles_T(np.asarray(inputs["Wq"], np.float32)),
        "wkt": wT_tiles_T(np.asarray(inputs["Wk"], np.float32)),
        "wvt": wT_tiles_N(np.asarray(inputs["Wv"], np.float32)),
        "wot": wT_tiles_N(np.asarray(inputs["Wo"], np.float32)),
    }
    b_cst = {
        "bqt": np.ascontiguousarray(
            np.asarray(inputs["bq"], np.float32).reshape(H, 64).T
        ),
        "bkt": np.ascontiguousarray(
            np.asarray(inputs["bk"], np.float32).reshape(H, 64).T
        ),
        "bv_b": np.asarray(inputs["bv"], np.float32).reshape(1, E).astype(bft),
        "bo_b": np.asarray(inputs["bo"], np.float32).reshape(1, E).astype(bft),
    }

    if _nc_cache is None:
        _nc_cache = build_bass()
    nc = _nc_cache

    in_maps = []
    for core in range(NCORES):
        b0 = core * BL
        x = query[:, b0 : b0 + BL, :]  # [T, BL, E]
        xr = np.ascontiguousarray(x.transpose(1, 0, 2).reshape(ROWS, E))
        xt = np.ascontiguousarray(
            xr.T.astype(bft).reshape(8, 128, ROWS).transpose(1, 0, 2)
        ).reshape(128, 8 * ROWS)
        in_maps.append(
            {
                "xt_t": xt,
                "kvt": kvt_all[b0 : b0 + BL],
                "maskt": np.ascontiguousarray(mm[b0 : b0 + BL]),
                **wt_tiles,
                **b_cst,
            }
        )

    res = run_bass_kernel_spmd(nc, in_maps, core_ids=list(range(NCORES)), trace=TRACE)
    LAST_RESULTS = res
    outs = []
    for core in range(NCORES):
        o = res.results[core]["out"].reshape(BL, T, E).transpose(1, 0, 2)
        outs.append(o)
    return np.concatenate(outs, axis=1).astype(np.float32)
